# revision 10
# baseline (speedup 1.0000x reference)
"""PointPillarScatter + 1x1 conv on 8 Trainium2 NeuronCores.

Strategy (per the spatial-sharding hint): shard the BEV grid along X across the
8 cores (64 x-rows each). The host buckets pillars by x-row (mask+drop of
invalid / out-of-shard pillars) and pads each row's pillars to a fixed number
of 128-wide slots. On device, for each output x-row:

  sel[v, y]    = (y_v == iota_y)          (DVE compare against an iota row)
  canvas[c, y] = f^T @ sel                (TensorE, k=pillar slots)
  out[o, y]    = wT^T @ canvas            (TensorE, k=channels)
  out += bias; PSUM -> SBUF               (ScalarE activation w/ bias)

so the scatter itself is realized as a selection matmul — no dense canvas in
DRAM, no indirect DMA. Results are packed two 8-row blocks per 128-partition
SBUF tile and written with 2 MB DMAs, once per batch element (the reference
broadcasts the same BEV image to every batch slice).
"""

import math

import numpy as np

X = 512
Y = 512
C = 64          # in/out channels
N_CORES = 8
XS = X // N_CORES  # x-rows per core


_PROGRAM_CACHE = {}


def _declare_io(nc, nsub, batch):
    import concourse.mybir as mybir

    f32 = mybir.dt.float32
    nq = XS * nsub
    tensors = {
        "feat": nc.dram_tensor("feat", [128, nq * C], f32, kind="ExternalInput"),
        "ysel": nc.dram_tensor("ysel", [128, nq], f32, kind="ExternalInput"),
        "iota": nc.dram_tensor("iota", [128, Y], f32, kind="ExternalInput"),
        "wt": nc.dram_tensor("wt", [C, C], f32, kind="ExternalInput"),
        "bias": nc.dram_tensor("bias", [C, 1], f32, kind="ExternalInput"),
    }
    return tensors


def _load_consts(nc, cpool, tensors, nsub):
    import concourse.mybir as mybir

    f32 = mybir.dt.float32
    nq = XS * nsub
    wt_t = cpool.tile([C, C], f32)
    nc.sync.dma_start(out=wt_t[:], in_=tensors["wt"].ap()[:])
    bias_t = cpool.tile([C, 1], f32)
    nc.sync.dma_start(out=bias_t[:], in_=tensors["bias"].ap()[:])
    iota_t = cpool.tile([128, Y], f32)
    nc.sync.dma_start(out=iota_t[:], in_=tensors["iota"].ap()[:])
    ys_t = cpool.tile([128, nq], f32)
    nc.sync.dma_start(out=ys_t[:], in_=tensors["ysel"].ap()[:])
    fbig = cpool.tile([128, nq * C], f32)
    nc.sync.dma_start(out=fbig[:], in_=tensors["feat"].ap()[:])
    return wt_t, bias_t, iota_t, ys_t, fbig


def _out_view(out_handle, b, g):
    """DRAM AP [64, 16*Y] for batch b, 16-x-row group g: partition = o channel,
    free = 16 x-rows * Y fully contiguous per partition (fast DMA path)."""
    import concourse.bass as bass

    ap = out_handle.ap()
    off = b * (C * XS * Y) + (16 * g) * Y
    return bass.AP(tensor=ap.tensor, offset=off,
                   ap=[[XS * Y, C], [1, 16 * Y]])


def _emit_body(nc, consts, pools, out_handle, nsub, batch):
    """One full per-core computation writing out_handle."""
    import concourse.mybir as mybir

    f32 = mybir.dt.float32
    wt_t, bias_t, iota_t, ys_t, fbig = consts
    wpool, ppool, opool = pools
    for g in range(4):
        ob = opool.tile([C, 16 * Y], f32, tag="ob")
        for slot in range(16):
            r = g * 16 + slot
            psum_c = ppool.tile([C, Y], f32, tag="pc")
            for j in range(nsub):
                q = r * nsub + j
                sel = wpool.tile([128, Y], f32, tag="sel")
                nc.vector.tensor_tensor(
                    out=sel[:],
                    in0=ys_t[:, q:q + 1].to_broadcast([128, Y]),
                    in1=iota_t[:],
                    op=mybir.AluOpType.is_equal,
                )
                nc.tensor.matmul(
                    psum_c[:],
                    lhsT=fbig[:, q * C:(q + 1) * C],
                    rhs=sel[:],
                    start=(j == 0),
                    stop=(j == nsub - 1),
                )
            sb_c = wpool.tile([C, Y], f32, tag="sbc")
            nc.vector.tensor_copy(out=sb_c[:], in_=psum_c[:])
            psum_o = ppool.tile([C, Y], f32, tag="po")
            nc.tensor.matmul(
                psum_o[:], lhsT=wt_t[:], rhs=sb_c[:], start=True, stop=True
            )
            nc.scalar.activation(
                out=ob[:, slot * Y:(slot + 1) * Y],
                in_=psum_o[:],
                func=mybir.ActivationFunctionType.Identity,
                bias=bias_t[:],
            )
        for b in range(batch):
            nc.sync.dma_start(out=_out_view(out_handle, b, g), in_=ob[:])


def _build_program(nsub: int, batch: int, n_rep: int = 1):
    """Build + compile the per-core Bass program (same program on all cores).

    n_rep > 1 emits independent replicas (each with its own output tensor) for
    slope-based timing; the graded path uses n_rep=1.
    """
    import concourse.bacc as bacc
    import concourse.mybir as mybir
    import concourse.tile as tile

    f32 = mybir.dt.float32
    nc = bacc.Bacc(
        "TRN2",
        target_bir_lowering=False,
        debug=False,
        enable_asserts=False,
        num_devices=N_CORES,
    )
    tensors = _declare_io(nc, nsub, batch)
    outs = [
        nc.dram_tensor("out" if n_rep == 1 else f"out{i}",
                       [batch, C, XS, Y], f32, kind="ExternalOutput")
        for i in range(n_rep)
    ]

    with tile.TileContext(nc) as tc:
        with (
            tc.tile_pool(name="const", bufs=1) as cpool,
            tc.tile_pool(name="work", bufs=4) as wpool,
            tc.tile_pool(name="psum", bufs=3, space="PSUM") as ppool,
            tc.tile_pool(name="obuf", bufs=2) as opool,
        ):
            consts = _load_consts(nc, cpool, tensors, nsub)
            for i in range(n_rep):
                _emit_body(nc, consts, (wpool, ppool, opool), outs[i], nsub, batch)
    nc.compile()
    return nc


def _prepare_inputs(pillar_features, voxel_indices, conv_w, conv_b, nsub):
    """Bucket pillars by global x-row; build per-core padded slot arrays."""
    pf = np.ascontiguousarray(pillar_features, dtype=np.float32)
    vi = np.asarray(voxel_indices)
    x = vi[:, 0].astype(np.int64)
    y = vi[:, 1].astype(np.int64)
    valid = (x >= 0) & (x < X) & (y >= 0) & (y < Y)
    sel_idx = np.nonzero(valid)[0]
    # de-duplicate cells, keeping the LAST occurrence (torch loop semantics:
    # last write wins). Actual reference inputs are unique, this is a safety
    # net; device-side the selection matmul would SUM duplicates instead.
    cell = x[sel_idx] * Y + y[sel_idx]
    last = np.full(X * Y, -1, np.int64)
    last[cell] = sel_idx  # later occurrences overwrite earlier ones
    keep = last[cell] == sel_idx
    sel_idx = sel_idx[keep]
    row = x[sel_idx]
    order = np.argsort(row, kind="stable")
    sel_idx = sel_idx[order]
    row = row[order]
    yv = y[sel_idx]

    counts = np.bincount(row, minlength=X)
    starts = np.zeros(X + 1, np.int64)
    np.cumsum(counts, out=starts[1:])
    pos = np.arange(len(sel_idx)) - starts[row]

    nq = XS * nsub
    core = row // XS
    r_local = row % XS
    j = pos // 128
    p = pos % 128
    q = r_local * nsub + j

    feat_arr = np.zeros((N_CORES, 128, nq, C), np.float32)
    feat_arr[core, p, q, :] = pf[sel_idx]
    ysel_arr = np.full((N_CORES, 128, nq), -1.0, np.float32)
    ysel_arr[core, p, q] = yv.astype(np.float32)

    iota_arr = np.ascontiguousarray(
        np.broadcast_to(np.arange(Y, dtype=np.float32), (128, Y))
    )
    wt_arr = np.ascontiguousarray(np.asarray(conv_w, np.float32).T)  # [c, o]
    bias_arr = np.ascontiguousarray(np.asarray(conv_b, np.float32).reshape(C, 1))

    in_maps = []
    for k in range(N_CORES):
        in_maps.append(
            {
                "feat": feat_arr[k].reshape(128, nq * C),
                "ysel": ysel_arr[k],
                "iota": iota_arr,
                "wt": wt_arr,
                "bias": bias_arr,
            }
        )
    max_count = int(counts.max()) if counts.size else 0
    return in_maps, max_count


def _required_nsub(voxel_indices):
    vi = np.asarray(voxel_indices)
    x = vi[:, 0].astype(np.int64)
    y = vi[:, 1].astype(np.int64)
    valid = (x >= 0) & (x < X) & (y >= 0) & (y < Y)
    if valid.any():
        max_count = int(np.bincount(x[valid], minlength=X).max())
    else:
        max_count = 0
    return max(1, math.ceil(max_count / 128))


def kernel(pillar_features, conv_w, conv_b, voxel_indices, batch_size):
    from concourse.bass_utils import run_bass_kernel_spmd

    batch = int(batch_size)
    nsub = _required_nsub(voxel_indices)

    key = (nsub, batch)
    if key not in _PROGRAM_CACHE:
        _PROGRAM_CACHE[key] = _build_program(nsub, batch)
    nc = _PROGRAM_CACHE[key]

    in_maps, _ = _prepare_inputs(pillar_features, voxel_indices, conv_w, conv_b, nsub)
    res = run_bass_kernel_spmd(nc, in_maps, core_ids=list(range(N_CORES)))

    full = np.empty((batch, C, X, Y), np.float32)
    for k in range(N_CORES):
        full[:, :, k * XS:(k + 1) * XS, :] = res.results[k]["out"]
    return full


# revision 12
# speedup vs baseline: 2.0545x; 2.0545x over previous
"""PointPillarScatter + 1x1 conv on 8 Trainium2 NeuronCores.

Strategy (per the spatial-sharding hint): shard the BEV grid along X across the
8 cores (64 x-rows each). The host buckets pillars by x-row (mask+drop of
invalid / out-of-shard pillars) and pads each row's pillars to a fixed number
of 128-wide slots. On device, for each output x-row:

  sel[v, y]    = (y_v == iota_y)          (DVE compare against an iota row)
  canvas[c, y] = f^T @ sel                (TensorE, k=pillar slots)
  out[o, y]    = wT^T @ canvas            (TensorE, k=channels)
  out += bias; PSUM -> SBUF               (ScalarE activation w/ bias)

so the scatter itself is realized as a selection matmul — no dense canvas in
DRAM, no indirect DMA. Results are packed two 8-row blocks per 128-partition
SBUF tile and written with 2 MB DMAs, once per batch element (the reference
broadcasts the same BEV image to every batch slice).
"""

import math

import numpy as np

X = 512
Y = 512
C = 64          # in/out channels
N_CORES = 8
XS = X // N_CORES  # x-rows per core


_PROGRAM_CACHE = {}


def _declare_io(nc, nsub, batch):
    import concourse.mybir as mybir

    f32 = mybir.dt.float32
    nq = XS * nsub
    tensors = {
        "feat": nc.dram_tensor("feat", [128, nq * C], f32, kind="ExternalInput"),
        "ysel": nc.dram_tensor("ysel", [128, nq], f32, kind="ExternalInput"),
        "iota": nc.dram_tensor("iota", [128, Y], f32, kind="ExternalInput"),
        "wt": nc.dram_tensor("wt", [C, C], f32, kind="ExternalInput"),
        "bias": nc.dram_tensor("bias", [C, 1], f32, kind="ExternalInput"),
    }
    return tensors


def _load_consts(nc, cpool, tensors, nsub):
    import concourse.mybir as mybir

    f32 = mybir.dt.float32
    nq = XS * nsub
    wt_t = cpool.tile([C, C], f32)
    nc.sync.dma_start(out=wt_t[:], in_=tensors["wt"].ap()[:])
    bias_t = cpool.tile([C, 1], f32)
    nc.sync.dma_start(out=bias_t[:], in_=tensors["bias"].ap()[:])
    iota_t = cpool.tile([128, Y], f32)
    nc.sync.dma_start(out=iota_t[:], in_=tensors["iota"].ap()[:])
    ys_t = cpool.tile([128, nq], f32)
    nc.sync.dma_start(out=ys_t[:], in_=tensors["ysel"].ap()[:])
    # feature slots loaded as 4 per-group chunks so group 0's matmuls can
    # start ~4x sooner than with one monolithic 2 MB load (one-shot ramp)
    fbig = cpool.tile([128, nq * C], f32)
    gq = (nq // 4) * C
    for g in range(4):
        nc.sync.dma_start(
            out=fbig[:, g * gq:(g + 1) * gq],
            in_=tensors["feat"].ap()[:, g * gq:(g + 1) * gq],
        )
    return wt_t, bias_t, iota_t, ys_t, fbig


def _out_view(out_handle, b, g):
    """DRAM AP [64, 16*Y] for batch b, 16-x-row group g: partition = o channel,
    free = 16 x-rows * Y fully contiguous per partition (fast DMA path)."""
    import concourse.bass as bass

    ap = out_handle.ap()
    off = b * (C * XS * Y) + (16 * g) * Y
    return bass.AP(tensor=ap.tensor, offset=off,
                   ap=[[XS * Y, C], [1, 16 * Y]])


def _emit_body(nc, consts, pools, out_handle, nsub, batch):
    """One full per-core computation writing out_handle."""
    import concourse.mybir as mybir

    f32 = mybir.dt.float32
    wt_t, bias_t, iota_t, ys_t, fbig = consts
    wpool, ppool, opool = pools
    for g in range(4):
        ob = opool.tile([C, 16 * Y], f32, tag="ob")
        for slot in range(16):
            r = g * 16 + slot
            psum_c = ppool.tile([C, Y], f32, tag="pc")
            for j in range(nsub):
                q = r * nsub + j
                sel = wpool.tile([128, Y], f32, tag="sel")
                nc.vector.tensor_tensor(
                    out=sel[:],
                    in0=ys_t[:, q:q + 1].to_broadcast([128, Y]),
                    in1=iota_t[:],
                    op=mybir.AluOpType.is_equal,
                )
                nc.tensor.matmul(
                    psum_c[:],
                    lhsT=fbig[:, q * C:(q + 1) * C],
                    rhs=sel[:],
                    start=(j == 0),
                    stop=(j == nsub - 1),
                )
            sb_c = wpool.tile([C, Y], f32, tag="sbc")
            nc.vector.tensor_copy(out=sb_c[:], in_=psum_c[:])
            psum_o = ppool.tile([C, Y], f32, tag="po")
            nc.tensor.matmul(
                psum_o[:], lhsT=wt_t[:], rhs=sb_c[:], start=True, stop=True
            )
            nc.scalar.activation(
                out=ob[:, slot * Y:(slot + 1) * Y],
                in_=psum_o[:],
                func=mybir.ActivationFunctionType.Identity,
                bias=bias_t[:],
            )
        if g < 3:
            for b in range(batch):
                nc.sync.dma_start(out=_out_view(out_handle, b, g), in_=ob[:])
        else:
            # final group: split writes in half so the first half commits while
            # slots 8-15 still compute — shrinks the kernel tail (one-shot)
            import concourse.bass as bass

            ap = out_handle.ap()
            for half in range(2):
                for b in range(batch):
                    off = b * (C * XS * Y) + (16 * g + 8 * half) * Y
                    v = bass.AP(tensor=ap.tensor, offset=off,
                                ap=[[XS * Y, C], [1, 8 * Y]])
                    nc.sync.dma_start(
                        out=v, in_=ob[:, half * 8 * Y:(half + 1) * 8 * Y]
                    )


def _build_program(nsub: int, batch: int, n_rep: int = 1):
    """Build + compile the per-core Bass program (same program on all cores).

    n_rep > 1 emits independent replicas (each with its own output tensor) for
    slope-based timing; the graded path uses n_rep=1.
    """
    import concourse.bacc as bacc
    import concourse.mybir as mybir
    import concourse.tile as tile

    f32 = mybir.dt.float32
    nc = bacc.Bacc(
        "TRN2",
        target_bir_lowering=False,
        debug=False,
        enable_asserts=False,
        num_devices=N_CORES,
    )
    tensors = _declare_io(nc, nsub, batch)
    outs = [
        nc.dram_tensor("out" if n_rep == 1 else f"out{i}",
                       [batch, C, XS, Y], f32, kind="ExternalOutput")
        for i in range(n_rep)
    ]

    with tile.TileContext(nc) as tc:
        with (
            tc.tile_pool(name="const", bufs=1) as cpool,
            tc.tile_pool(name="work", bufs=4) as wpool,
            tc.tile_pool(name="psum", bufs=3, space="PSUM") as ppool,
            tc.tile_pool(name="obuf", bufs=2) as opool,
        ):
            consts = _load_consts(nc, cpool, tensors, nsub)
            for i in range(n_rep):
                _emit_body(nc, consts, (wpool, ppool, opool), outs[i], nsub, batch)
    nc.compile()
    return nc


def _prepare_inputs(pillar_features, voxel_indices, conv_w, conv_b, nsub):
    """Bucket pillars by global x-row; build per-core padded slot arrays."""
    pf = np.ascontiguousarray(pillar_features, dtype=np.float32)
    vi = np.asarray(voxel_indices)
    x = vi[:, 0].astype(np.int64)
    y = vi[:, 1].astype(np.int64)
    valid = (x >= 0) & (x < X) & (y >= 0) & (y < Y)
    sel_idx = np.nonzero(valid)[0]
    # de-duplicate cells, keeping the LAST occurrence (torch loop semantics:
    # last write wins). Actual reference inputs are unique, this is a safety
    # net; device-side the selection matmul would SUM duplicates instead.
    cell = x[sel_idx] * Y + y[sel_idx]
    last = np.full(X * Y, -1, np.int64)
    last[cell] = sel_idx  # later occurrences overwrite earlier ones
    keep = last[cell] == sel_idx
    sel_idx = sel_idx[keep]
    row = x[sel_idx]
    order = np.argsort(row, kind="stable")
    sel_idx = sel_idx[order]
    row = row[order]
    yv = y[sel_idx]

    counts = np.bincount(row, minlength=X)
    starts = np.zeros(X + 1, np.int64)
    np.cumsum(counts, out=starts[1:])
    pos = np.arange(len(sel_idx)) - starts[row]

    nq = XS * nsub
    core = row // XS
    r_local = row % XS
    j = pos // 128
    p = pos % 128
    q = r_local * nsub + j

    feat_arr = np.zeros((N_CORES, 128, nq, C), np.float32)
    feat_arr[core, p, q, :] = pf[sel_idx]
    ysel_arr = np.full((N_CORES, 128, nq), -1.0, np.float32)
    ysel_arr[core, p, q] = yv.astype(np.float32)

    iota_arr = np.ascontiguousarray(
        np.broadcast_to(np.arange(Y, dtype=np.float32), (128, Y))
    )
    wt_arr = np.ascontiguousarray(np.asarray(conv_w, np.float32).T)  # [c, o]
    bias_arr = np.ascontiguousarray(np.asarray(conv_b, np.float32).reshape(C, 1))

    in_maps = []
    for k in range(N_CORES):
        in_maps.append(
            {
                "feat": feat_arr[k].reshape(128, nq * C),
                "ysel": ysel_arr[k],
                "iota": iota_arr,
                "wt": wt_arr,
                "bias": bias_arr,
            }
        )
    max_count = int(counts.max()) if counts.size else 0
    return in_maps, max_count


def _required_nsub(voxel_indices):
    vi = np.asarray(voxel_indices)
    x = vi[:, 0].astype(np.int64)
    y = vi[:, 1].astype(np.int64)
    valid = (x >= 0) & (x < X) & (y >= 0) & (y < Y)
    if valid.any():
        max_count = int(np.bincount(x[valid], minlength=X).max())
    else:
        max_count = 0
    return max(1, math.ceil(max_count / 128))


def kernel(pillar_features, conv_w, conv_b, voxel_indices, batch_size):
    from concourse.bass_utils import run_bass_kernel_spmd

    batch = int(batch_size)
    nsub = _required_nsub(voxel_indices)

    key = (nsub, batch)
    if key not in _PROGRAM_CACHE:
        _PROGRAM_CACHE[key] = _build_program(nsub, batch)
    nc = _PROGRAM_CACHE[key]

    in_maps, _ = _prepare_inputs(pillar_features, voxel_indices, conv_w, conv_b, nsub)
    res = run_bass_kernel_spmd(nc, in_maps, core_ids=list(range(N_CORES)))

    full = np.empty((batch, C, X, Y), np.float32)
    for k in range(N_CORES):
        full[:, :, k * XS:(k + 1) * XS, :] = res.results[k]["out"]
    return full


# revision 13
# speedup vs baseline: 34.0941x; 16.5945x over previous
"""PointPillarScatter + 1x1 conv on 8 Trainium2 NeuronCores.

Strategy (per the spatial-sharding hint): shard the BEV grid along X across the
8 cores (64 x-rows each). The host buckets pillars by x-row (mask+drop of
invalid / out-of-shard pillars) and pads each row's pillars to a fixed number
of 128-wide slots. On device, for each output x-row:

  sel[v, y]    = (y_v == iota_y)          (DVE compare against an iota row)
  canvas[c, y] = f^T @ sel                (TensorE, k=pillar slots)
  out[o, y]    = wT^T @ canvas            (TensorE, k=channels)
  out += bias; PSUM -> SBUF               (ScalarE activation w/ bias)

so the scatter itself is realized as a selection matmul — no dense canvas in
DRAM, no indirect DMA. Results are packed two 8-row blocks per 128-partition
SBUF tile and written with 2 MB DMAs, once per batch element (the reference
broadcasts the same BEV image to every batch slice).
"""

import math

import numpy as np

X = 512
Y = 512
C = 64          # in/out channels
N_CORES = 8
XS = X // N_CORES  # x-rows per core


_PROGRAM_CACHE = {}


def _declare_io(nc, nsub, batch):
    import concourse.mybir as mybir

    f32 = mybir.dt.float32
    nq = XS * nsub
    tensors = {
        "feat": nc.dram_tensor("feat", [128, nq * C], f32, kind="ExternalInput"),
        "ysel": nc.dram_tensor("ysel", [128, nq], f32, kind="ExternalInput"),
        "iota": nc.dram_tensor("iota", [128, Y], f32, kind="ExternalInput"),
        "wt": nc.dram_tensor("wt", [C, C], f32, kind="ExternalInput"),
        "bias": nc.dram_tensor("bias", [C, 1], f32, kind="ExternalInput"),
    }
    return tensors


def _load_consts(nc, cpool, tensors, nsub):
    import concourse.mybir as mybir

    f32 = mybir.dt.float32
    nq = XS * nsub
    wt_t = cpool.tile([C, C], f32)
    nc.sync.dma_start(out=wt_t[:], in_=tensors["wt"].ap()[:])
    bias_t = cpool.tile([C, 1], f32)
    nc.sync.dma_start(out=bias_t[:], in_=tensors["bias"].ap()[:])
    iota_t = cpool.tile([128, Y], f32)
    nc.sync.dma_start(out=iota_t[:], in_=tensors["iota"].ap()[:])
    ys_t = cpool.tile([128, nq], f32)
    nc.sync.dma_start(out=ys_t[:], in_=tensors["ysel"].ap()[:])
    # feature slots loaded as 4 per-group chunks so group 0's matmuls can
    # start ~4x sooner than with one monolithic 2 MB load (one-shot ramp)
    fbig = cpool.tile([128, nq * C], f32)
    gq = (nq // 4) * C
    for g in range(4):
        nc.sync.dma_start(
            out=fbig[:, g * gq:(g + 1) * gq],
            in_=tensors["feat"].ap()[:, g * gq:(g + 1) * gq],
        )
    return wt_t, bias_t, iota_t, ys_t, fbig


def _out_view(out_handle, b, g):
    """DRAM AP [64, 16*Y] for batch b, 16-x-row group g: partition = o channel,
    free = 16 x-rows * Y fully contiguous per partition (fast DMA path)."""
    import concourse.bass as bass

    ap = out_handle.ap()
    off = b * (C * XS * Y) + (16 * g) * Y
    return bass.AP(tensor=ap.tensor, offset=off,
                   ap=[[XS * Y, C], [1, 16 * Y]])


def _emit_body(nc, consts, pools, out_handle, nsub, batch):
    """One full per-core computation writing out_handle."""
    import concourse.mybir as mybir

    f32 = mybir.dt.float32
    wt_t, bias_t, iota_t, ys_t, fbig = consts
    wpool, ppool, opool = pools
    for g in range(4):
        ob = opool.tile([C, 16 * Y], f32, tag="ob")
        for slot in range(16):
            r = g * 16 + slot
            psum_c = ppool.tile([C, Y], f32, tag="pc")
            for j in range(nsub):
                q = r * nsub + j
                sel = wpool.tile([128, Y], f32, tag="sel")
                nc.vector.tensor_tensor(
                    out=sel[:],
                    in0=ys_t[:, q:q + 1].to_broadcast([128, Y]),
                    in1=iota_t[:],
                    op=mybir.AluOpType.is_equal,
                )
                nc.tensor.matmul(
                    psum_c[:],
                    lhsT=fbig[:, q * C:(q + 1) * C],
                    rhs=sel[:],
                    start=(j == 0),
                    stop=(j == nsub - 1),
                )
            sb_c = wpool.tile([C, Y], f32, tag="sbc")
            nc.vector.tensor_copy(out=sb_c[:], in_=psum_c[:])
            psum_o = ppool.tile([C, Y], f32, tag="po")
            nc.tensor.matmul(
                psum_o[:], lhsT=wt_t[:], rhs=sb_c[:], start=True, stop=True
            )
            nc.scalar.activation(
                out=ob[:, slot * Y:(slot + 1) * Y],
                in_=psum_o[:],
                func=mybir.ActivationFunctionType.Identity,
                bias=bias_t[:],
            )
        if g < 3:
            for b in range(batch):
                nc.sync.dma_start(out=_out_view(out_handle, b, g), in_=ob[:])
        else:
            # final group: split writes in half so the first half commits while
            # slots 8-15 still compute — shrinks the kernel tail (one-shot)
            import concourse.bass as bass

            ap = out_handle.ap()
            for half in range(2):
                for b in range(batch):
                    off = b * (C * XS * Y) + (16 * g + 8 * half) * Y
                    v = bass.AP(tensor=ap.tensor, offset=off,
                                ap=[[XS * Y, C], [1, 8 * Y]])
                    nc.sync.dma_start(
                        out=v, in_=ob[:, half * 8 * Y:(half + 1) * 8 * Y]
                    )


def _build_program(nsub: int, batch: int, n_rep: int = 1):
    """Build + compile the per-core Bass program (same program on all cores).

    n_rep > 1 emits independent replicas (each with its own output tensor) for
    slope-based timing; the graded path uses n_rep=1.
    """
    import concourse.bacc as bacc
    import concourse.mybir as mybir
    import concourse.tile as tile

    f32 = mybir.dt.float32
    nc = bacc.Bacc(
        "TRN2",
        target_bir_lowering=False,
        debug=False,
        enable_asserts=False,
        num_devices=N_CORES,
    )
    tensors = _declare_io(nc, nsub, batch)
    outs = [
        nc.dram_tensor("out" if n_rep == 1 else f"out{i}",
                       [batch, C, XS, Y], f32, kind="ExternalOutput")
        for i in range(n_rep)
    ]

    with tile.TileContext(nc) as tc:
        with (
            tc.tile_pool(name="const", bufs=1) as cpool,
            tc.tile_pool(name="work", bufs=4) as wpool,
            tc.tile_pool(name="psum", bufs=3, space="PSUM") as ppool,
            tc.tile_pool(name="obuf", bufs=3) as opool,
        ):
            consts = _load_consts(nc, cpool, tensors, nsub)
            for i in range(n_rep):
                _emit_body(nc, consts, (wpool, ppool, opool), outs[i], nsub, batch)
    nc.compile()
    return nc


def _prepare_inputs(pillar_features, voxel_indices, conv_w, conv_b, nsub):
    """Bucket pillars by global x-row; build per-core padded slot arrays."""
    pf = np.ascontiguousarray(pillar_features, dtype=np.float32)
    vi = np.asarray(voxel_indices)
    x = vi[:, 0].astype(np.int64)
    y = vi[:, 1].astype(np.int64)
    valid = (x >= 0) & (x < X) & (y >= 0) & (y < Y)
    sel_idx = np.nonzero(valid)[0]
    # de-duplicate cells, keeping the LAST occurrence (torch loop semantics:
    # last write wins). Actual reference inputs are unique, this is a safety
    # net; device-side the selection matmul would SUM duplicates instead.
    cell = x[sel_idx] * Y + y[sel_idx]
    last = np.full(X * Y, -1, np.int64)
    last[cell] = sel_idx  # later occurrences overwrite earlier ones
    keep = last[cell] == sel_idx
    sel_idx = sel_idx[keep]
    row = x[sel_idx]
    order = np.argsort(row, kind="stable")
    sel_idx = sel_idx[order]
    row = row[order]
    yv = y[sel_idx]

    counts = np.bincount(row, minlength=X)
    starts = np.zeros(X + 1, np.int64)
    np.cumsum(counts, out=starts[1:])
    pos = np.arange(len(sel_idx)) - starts[row]

    nq = XS * nsub
    core = row // XS
    r_local = row % XS
    j = pos // 128
    p = pos % 128
    q = r_local * nsub + j

    feat_arr = np.zeros((N_CORES, 128, nq, C), np.float32)
    feat_arr[core, p, q, :] = pf[sel_idx]
    ysel_arr = np.full((N_CORES, 128, nq), -1.0, np.float32)
    ysel_arr[core, p, q] = yv.astype(np.float32)

    iota_arr = np.ascontiguousarray(
        np.broadcast_to(np.arange(Y, dtype=np.float32), (128, Y))
    )
    wt_arr = np.ascontiguousarray(np.asarray(conv_w, np.float32).T)  # [c, o]
    bias_arr = np.ascontiguousarray(np.asarray(conv_b, np.float32).reshape(C, 1))

    in_maps = []
    for k in range(N_CORES):
        in_maps.append(
            {
                "feat": feat_arr[k].reshape(128, nq * C),
                "ysel": ysel_arr[k],
                "iota": iota_arr,
                "wt": wt_arr,
                "bias": bias_arr,
            }
        )
    max_count = int(counts.max()) if counts.size else 0
    return in_maps, max_count


def _required_nsub(voxel_indices):
    vi = np.asarray(voxel_indices)
    x = vi[:, 0].astype(np.int64)
    y = vi[:, 1].astype(np.int64)
    valid = (x >= 0) & (x < X) & (y >= 0) & (y < Y)
    if valid.any():
        max_count = int(np.bincount(x[valid], minlength=X).max())
    else:
        max_count = 0
    return max(1, math.ceil(max_count / 128))


def kernel(pillar_features, conv_w, conv_b, voxel_indices, batch_size):
    from concourse.bass_utils import run_bass_kernel_spmd

    batch = int(batch_size)
    nsub = _required_nsub(voxel_indices)

    key = (nsub, batch)
    if key not in _PROGRAM_CACHE:
        _PROGRAM_CACHE[key] = _build_program(nsub, batch)
    nc = _PROGRAM_CACHE[key]

    in_maps, _ = _prepare_inputs(pillar_features, voxel_indices, conv_w, conv_b, nsub)
    res = run_bass_kernel_spmd(nc, in_maps, core_ids=list(range(N_CORES)))

    full = np.empty((batch, C, X, Y), np.float32)
    for k in range(N_CORES):
        full[:, :, k * XS:(k + 1) * XS, :] = res.results[k]["out"]
    return full
